# revision 1
# baseline (speedup 1.0000x reference)
"""GroupedQueryAttention Trainium2 kernel (8 NeuronCores).

Sharding: core c -> (batch b = c//4, kv-group g = c%4). Each core computes
the 4 heads of its kv-group for its batch (tensor parallel over head groups,
data parallel over batch). Attention outputs (transposed, [head*HD, L]) are
AllGather-ed among the 4 cores of each batch, after which every core computes
a disjoint 512-column slice of the output projection. The host concatenates
the 8 column-slices - no cross-core reduction needed.

Layout trick: x is fed pre-transposed ([D, L]) so x^T tiles serve as the
stationary operand producing q/k/v in natural [L, hd] layout, where rmsnorm
(free-dim reduce) and rope (free-dim half-swap) are cheap on DVE. q/k are
then PE-transposed to [hd, L] to feed the scores matmul. Scores are computed
directly transposed ([key, query]) so the AV matmul needs no transpose of the
probabilities; softmax row-sums come from a ones-column matmul accumulated
alongside AV. No max-subtraction is needed: |scores|/HD^2 <= 128/16384 by
Cauchy-Schwarz (q,k are rms-normalized), so exp() is always well-conditioned.

All matmuls run as float32r (full PE rate at moving-dim >= 256).

Perf notes (TimelineSim, collective stubbed as DMA): ~374 us/core.
PE busy floor is ~250 us (proj 82 + attn 103 + out-proj 55 + transposes).
Probed: moving exp off ACT onto DVE makes it worse (441 us) - the kernel
is PE/dependency-paced, not ACT-starved. Next levers would be fp8
DoubleRow on the two big projections (~-50 us PE, accuracy risk) or
restructuring the scores->exp->AV chain to shorten the critical path.
"""

import numpy as np

import concourse.bacc as bacc
import concourse.bass as bass
import concourse.tile as tile
from concourse import mybir
from concourse.bass_utils import run_bass_kernel_spmd

F32 = mybir.dt.float32
F32R = mybir.dt.float32r
AF = mybir.ActivationFunctionType
ALU = mybir.AluOpType

B, L, D = 2, 2048, 2048
H, G, HD = 16, 4, 128
GS = H // G  # heads per kv group = 4
NCORES = 8
CHUNK = 512  # query-chunk (psum bank width in f32)
NLT = L // 128  # 16 row-tiles
NDK = D // 128  # 16 contraction-tiles
NCH = L // CHUNK  # 4 query chunks
EPS = 1e-6
SM_SCALE = 1.0 / float(HD * HD)

REPLICA_GROUPS = [[0, 1, 2, 3], [4, 5, 6, 7]]

_CACHE = {}
LAST_RESULT = None  # BassKernelResults of the most recent run (for test harness)


def _r(ap):
    return ap.bitcast(F32R)


def _build_bass(sim_mode=False):
    # Bacc (not raw Bass): its compile() runs move_matmul_waits_to_ldweights
    # + generate_event_semaphores, required to satisfy the 1-wait-per-
    # instruction hardware constraint that walrus enforces.
    nc = bacc.Bacc("TRN2", target_bir_lowering=False, debug=False)

    xT = nc.declare_dram_parameter("xT", [D, L], F32, isOutput=False)
    wq = nc.declare_dram_parameter("wq", [D, GS * HD], F32, isOutput=False)
    wkv = nc.declare_dram_parameter("wkv", [D, 2 * HD], F32, isOutput=False)
    wo = nc.declare_dram_parameter("wo", [H * HD, CHUNK], F32, isOutput=False)
    cosq = nc.declare_dram_parameter("cosq", [L, GS * HD], F32, isOutput=False)
    sinq = nc.declare_dram_parameter("sinq", [L, GS * HD], F32, isOutput=False)
    cosk = nc.declare_dram_parameter("cosk", [L, HD], F32, isOutput=False)
    sink = nc.declare_dram_parameter("sink", [L, HD], F32, isOutput=False)
    maskd = nc.declare_dram_parameter("maskd", [CHUNK, CHUNK], F32, isOutput=False)
    ident = nc.declare_dram_parameter("ident", [128, 128], F32, isOutput=False)
    ones_col = nc.declare_dram_parameter("ones_col", [128, 1], F32, isOutput=False)
    ones_row = nc.declare_dram_parameter("ones_row", [1, 128], F32, isOutput=False)
    out = nc.declare_dram_parameter("out", [L, CHUNK], F32, isOutput=True)

    # [p, t, cols] views (partition = row within 128-tile)
    xT_v = xT[:].rearrange("(t p) l -> p t l", p=128)
    wq_v = wq[:].rearrange("(t p) n -> p t n", p=128)
    wkv_v = wkv[:].rearrange("(t p) n -> p t n", p=128)
    wo_v = wo[:].rearrange("(t p) n -> p t n", p=128)
    cosq_v = cosq[:].rearrange("(t p) n -> p t n", p=128)
    sinq_v = sinq[:].rearrange("(t p) n -> p t n", p=128)
    cosk_v = cosk[:].rearrange("(t p) n -> p t n", p=128)
    sink_v = sink[:].rearrange("(t p) n -> p t n", p=128)
    maskd_v = maskd[:].rearrange("(t p) n -> p t n", p=128)

    with tile.TileContext(nc) as tc:
        with (
            tc.tile_pool(name="persist", bufs=1) as persist,
            tc.tile_pool(name="consts", bufs=1) as consts,
            tc.tile_pool(name="cc", bufs=2, space="DRAM") as ccpool,
        ):
            # persistent SBUF
            qT_sb = persist.tile([128, GS, L], F32R)  # 4 MB, [hd, head, l]
            kT_sb = persist.tile([128, L], F32R)  # 1 MB, [hd, l]
            v_sb = persist.tile([128, NLT, HD], F32R)  # 1 MB, [l, lt, hd]

            ident_sb = consts.tile([128, 128], F32)
            ones_col_sb = consts.tile([128, 1], F32R)
            ones_row_sb = consts.tile([1, 128], F32R)
            eps_sb = consts.tile([128, 1], F32)
            nc.gpsimd.memset(eps_sb[:], EPS)
            maskd_sb = consts.tile([128, NCH, CHUNK], F32)  # 1 MB
            nc.sync.dma_start(ident_sb[:], ident[:])
            nc.sync.dma_start(ones_col_sb[:], ones_col[:].bitcast(F32R))
            nc.sync.dma_start(ones_row_sb[:], ones_row[:].bitcast(F32R))
            nc.sync.dma_start(maskd_sb[:], maskd_v)

            # ---------------- Phase A: projections + rmsnorm + rope ---------
            with (
                tc.tile_pool(name="wts", bufs=1) as wts,
                tc.tile_pool(name="xin", bufs=3) as xin,
                tc.tile_pool(name="trig", bufs=3) as trig,
                tc.tile_pool(name="scrA", bufs=2) as scrA,
                tc.tile_pool(name="psA_q", bufs=2, space="PSUM") as psA_q,
                tc.tile_pool(name="psA_kv", bufs=2, space="PSUM") as psA_kv,
                tc.tile_pool(name="psA_tq", bufs=2, space="PSUM") as psA_tq,
                tc.tile_pool(name="psA_tk", bufs=2, space="PSUM") as psA_tk,
            ):
                wq_sb = wts.tile([128, NDK, GS * HD], F32R)  # 4 MB
                wkv_sb = wts.tile([128, NDK, 2 * HD], F32R)  # 2 MB
                nc.sync.dma_start(wq_sb[:], wq_v.bitcast(F32R))
                nc.sync.dma_start(wkv_sb[:], wkv_v.bitcast(F32R))

                for lt in range(NLT):
                    ls = slice(lt * 128, (lt + 1) * 128)
                    xt = xin.tile([128, NDK, 128], F32R, tag="xt")
                    nc.sync.dma_start(xt[:], xT_v[:, :, ls].bitcast(F32R))

                    cq_t = trig.tile([128, GS * HD], F32, tag="cq")
                    sq_t = trig.tile([128, GS * HD], F32, tag="sq")
                    ck_t = trig.tile([128, HD], F32, tag="ck")
                    sk_t = trig.tile([128, HD], F32, tag="sk")
                    nc.sync.dma_start(cq_t[:], cosq_v[:, lt, :])
                    nc.sync.dma_start(sq_t[:], sinq_v[:, lt, :])
                    nc.sync.dma_start(ck_t[:], cosk_v[:, lt, :])
                    nc.sync.dma_start(sk_t[:], sink_v[:, lt, :])

                    q_ps = psA_q.tile([128, GS * HD], F32, tag="q")
                    kv_ps = psA_kv.tile([128, 2 * HD], F32, tag="kv")
                    for dk in range(NDK):
                        nc.tensor.matmul(
                            q_ps[:], xt[:, dk, :], wq_sb[:, dk, :],
                            start=(dk == 0), stop=(dk == NDK - 1),
                        )
                        nc.tensor.matmul(
                            kv_ps[:], xt[:, dk, :], wkv_sb[:, dk, :],
                            start=(dk == 0), stop=(dk == NDK - 1),
                        )

                    # copy out of PSUM first (DVE reads at most 1 PSUM input)
                    qsb = scrA.tile([128, GS * HD], F32, tag="qsb")
                    kvsb = scrA.tile([128, 2 * HD], F32, tag="kvsb")
                    nc.vector.tensor_copy(qsb[:], q_ps[:])
                    nc.vector.tensor_copy(kvsb[:], kv_ps[:])
                    # rmsnorm stats (free-dim reduce per head)
                    sq_full = scrA.tile([128, GS * HD], F32, tag="sqf")
                    sums = scrA.tile([128, 8], F32, tag="sums")
                    rms = scrA.tile([128, 8], F32, tag="rms")
                    recip = scrA.tile([128, 8], F32, tag="recip")
                    nc.vector.tensor_mul(sq_full[:], qsb[:], qsb[:])
                    nc.vector.reduce_sum(
                        sums[:, 0:GS],
                        sq_full[:].rearrange("p (h d) -> p h d", h=GS),
                        axis=mybir.AxisListType.X,
                    )
                    sq_k = scrA.tile([128, HD], F32, tag="sqk")
                    nc.vector.tensor_mul(sq_k[:], kvsb[:, 0:HD], kvsb[:, 0:HD])
                    nc.vector.reduce_sum(
                        sums[:, GS:GS + 1], sq_k[:], axis=mybir.AxisListType.X
                    )
                    nc.scalar.activation(
                        rms[:, 0:GS + 1], sums[:, 0:GS + 1], AF.Sqrt,
                        scale=1.0 / HD, bias=eps_sb[:],
                    )
                    nc.vector.reciprocal(recip[:, 0:GS + 1], rms[:, 0:GS + 1])

                    # normalize (q_scale/k_scale are baked into cos/sin tables)
                    qn = scrA.tile([128, GS * HD], F32, tag="qn")
                    for h in range(GS):
                        hs = slice(h * HD, (h + 1) * HD)
                        nc.vector.tensor_scalar_mul(
                            qn[:, hs], qsb[:, hs], recip[:, h:h + 1]
                        )
                    kn = scrA.tile([128, HD], F32, tag="kn")
                    nc.vector.tensor_scalar_mul(
                        kn[:], kvsb[:, 0:HD], recip[:, GS:GS + 1]
                    )

                    # rope: qr = qn*cos' + swap_halves(qn)*sin'  (sign in sin')
                    t1q = scrA.tile([128, GS * HD], F32, tag="t1q")
                    t2q = scrA.tile([128, GS * HD], F32, tag="t2q")
                    nc.vector.tensor_mul(t1q[:], qn[:], cq_t[:])
                    qn3 = qn[:].rearrange("p (h d) -> p h d", h=GS)
                    t23 = t2q[:].rearrange("p (h d) -> p h d", h=GS)
                    sq3 = sq_t[:].rearrange("p (h d) -> p h d", h=GS)
                    hh = HD // 2
                    nc.vector.tensor_mul(
                        t23[:, :, 0:hh], qn3[:, :, hh:HD], sq3[:, :, 0:hh]
                    )
                    nc.vector.tensor_mul(
                        t23[:, :, hh:HD], qn3[:, :, 0:hh], sq3[:, :, hh:HD]
                    )
                    nc.vector.tensor_add(t1q[:], t1q[:], t2q[:])

                    t1k = scrA.tile([128, HD], F32, tag="t1k")
                    t2k = scrA.tile([128, HD], F32, tag="t2k")
                    nc.vector.tensor_mul(t1k[:], kn[:], ck_t[:])
                    nc.vector.tensor_mul(t2k[:, 0:hh], kn[:, hh:HD], sk_t[:, 0:hh])
                    nc.vector.tensor_mul(t2k[:, hh:HD], kn[:, 0:hh], sk_t[:, hh:HD])
                    nc.vector.tensor_add(t1k[:], t1k[:], t2k[:])

                    # transpose q/k to [hd, l] (v stays natural)
                    tq_ps = psA_tq.tile([128, GS * HD], F32, tag="tq")
                    for h in range(GS):
                        hs = slice(h * HD, (h + 1) * HD)
                        nc.tensor.transpose(
                            tq_ps[:, hs], t1q[:, hs], ident_sb[:]
                        )
                    nc.vector.tensor_copy(
                        qT_sb[:, :, ls],
                        tq_ps[:].rearrange("p (h d) -> p h d", h=GS),
                    )
                    tk_ps = psA_tk.tile([128, HD], F32, tag="tk")
                    nc.tensor.transpose(tk_ps[:], t1k[:], ident_sb[:])
                    nc.vector.tensor_copy(kT_sb[:, ls], tk_ps[:])
                    nc.vector.tensor_copy(v_sb[:, lt, :], kvsb[:, HD:2 * HD])

            # ---------------- Phase B: attention + per-chunk AllGather ------
            ag_outs = []
            with (
                tc.tile_pool(name="woP", bufs=1) as wopool,
                tc.tile_pool(name="wT", bufs=6) as wTpool,
                tc.tile_pool(name="attn", bufs=3) as attnpool,
                tc.tile_pool(name="scrB", bufs=2) as scrB,
                tc.tile_pool(name="psB_s", bufs=3, space="PSUM") as psB_s,
                tc.tile_pool(name="psB_a", bufs=2, space="PSUM") as psB_a,
                tc.tile_pool(name="psB_m", bufs=1, space="PSUM") as psB_m,
                tc.tile_pool(name="psB_b", bufs=1, space="PSUM") as psB_b,
                tc.tile_pool(name="psC", bufs=1, space="PSUM") as psC,
                tc.tile_pool(name="agin", bufs=3) as aginpool,
                tc.tile_pool(name="outsb", bufs=2) as outpool,
            ):
                wo_sb = wopool.tile([128, H, CHUNK], F32R)  # 4 MB (prefetch)
                nc.sync.dma_start(wo_sb[:], wo_v.bitcast(F32R))

                for c in range(NCH):
                    cs = slice(c * CHUNK, (c + 1) * CHUNK)
                    attn_my = ccpool.tile([GS * HD, CHUNK], F32, tag="attn_my")
                    for h in range(GS):
                        njt = 4 * (c + 1)  # causal: key tiles 0 .. 4c+3
                        a_ps = psB_a.tile([128, CHUNK], F32, tag="a")
                        m_ps = psB_m.tile([1, CHUNK], F32, tag="m")
                        for jt in range(njt):
                            js = slice(jt * 128, (jt + 1) * 128)
                            s_ps = psB_s.tile([128, CHUNK], F32, tag="s")
                            nc.tensor.matmul(
                                s_ps[:], kT_sb[:, js], qT_sb[:, h, cs],
                            )
                            wT = wTpool.tile([128, CHUNK], F32R, tag="w")
                            nc.scalar.activation(
                                wT[:], s_ps[:], AF.Exp, scale=SM_SCALE
                            )
                            jd = jt - 4 * c
                            if jd >= 0:  # diagonal band: apply causal mask
                                nc.vector.tensor_mul(
                                    wT[:], wT[:], maskd_sb[:, jd, :].bitcast(F32R)
                                )
                            nc.tensor.matmul(
                                a_ps[:], v_sb[:, jt, :], wT[:],
                                start=(jt == 0), stop=(jt == njt - 1),
                            )
                            nc.tensor.matmul(
                                m_ps[:], ones_col_sb[:], wT[:],
                                start=(jt == 0), stop=(jt == njt - 1),
                            )
                        # normalize: attnT_n = attnT * (1/rowsum) broadcast
                        rec = scrB.tile([1, CHUNK], F32R, tag="rec")
                        with nc.allow_low_precision(
                            reason="f32r rounding of softmax recip-sums"
                        ):
                            nc.vector.reciprocal(rec[:], m_ps[:])
                        b_ps = psB_b.tile([128, CHUNK], F32, tag="b")
                        nc.tensor.matmul(b_ps[:], ones_row_sb[:], rec[:])
                        b_sb = scrB.tile([128, CHUNK], F32, tag="bsb")
                        nc.vector.tensor_copy(b_sb[:], b_ps[:])
                        a_n = attnpool.tile([128, CHUNK], F32, tag="an")
                        nc.vector.tensor_mul(a_n[:], a_ps[:], b_sb[:])
                        nc.sync.dma_start(
                            attn_my[h * HD:(h + 1) * HD, :], a_n[:]
                        )
                    # NB: Shared addr_space is rejected for 4-core groups;
                    # Local HBM-HBM AllGather is supported (slightly slower).
                    ag_out = ccpool.tile([H * HD, CHUNK], F32, tag="ag_out")
                    if sim_mode:
                        nc.sync.dma_start(
                            ag_out[0:GS * HD, :], attn_my[:]
                        )
                        nc.sync.dma_start(
                            ag_out[GS * HD:2 * GS * HD, :], attn_my[:]
                        )
                        nc.sync.dma_start(
                            ag_out[2 * GS * HD:3 * GS * HD, :], attn_my[:]
                        )
                        nc.sync.dma_start(
                            ag_out[3 * GS * HD:4 * GS * HD, :], attn_my[:]
                        )
                    else:
                        nc.gpsimd.collective_compute(
                            "AllGather",
                            ALU.bypass,
                            ins=[attn_my.opt()],
                            outs=[ag_out.opt()],
                            replica_groups=REPLICA_GROUPS,
                        )
                    ag_outs.append(ag_out)

                # ------------ Phase C: output projection (my 512 columns) ---
                for c in range(NCH):
                    ag_v = ag_outs[c][:].rearrange("(t p) n -> p t n", p=128)
                    for it in range(NCH):
                        its = slice(it * 128, (it + 1) * 128)
                        ag_sb = aginpool.tile([128, H, 128], F32R, tag="ag")
                        nc.sync.dma_start(ag_sb[:], ag_v[:, :, its].bitcast(F32R))
                        o_ps = psC.tile([128, CHUNK], F32, tag="o")
                        for t in range(H):
                            nc.tensor.matmul(
                                o_ps[:], ag_sb[:, t, :], wo_sb[:, t, :],
                                start=(t == 0), stop=(t == H - 1),
                            )
                        o_sb = outpool.tile([128, CHUNK], F32, tag="o_sb")
                        nc.vector.tensor_copy(o_sb[:], o_ps[:])
                        nc.sync.dma_start(out[c * CHUNK + it * 128:
                                              c * CHUNK + (it + 1) * 128, :],
                                          o_sb[:])
    nc.compile()
    return nc


def _get_nc():
    if "nc" not in _CACHE:
        _CACHE["nc"] = _build_bass()
    return _CACHE["nc"]


def kernel(x, Wq, Wk, Wv, Wo, q_scale, k_scale, cos, sin, mask):
    global LAST_RESULT
    nc = _get_nc()

    f32 = np.float32
    x = np.asarray(x, f32)
    cos = np.asarray(cos, f32)
    sin = np.asarray(sin, f32)
    q_scale = np.asarray(q_scale, f32)
    k_scale = np.asarray(k_scale, f32)

    sgn = np.concatenate([-np.ones(HD // 2, f32), np.ones(HD // 2, f32)])
    qs_swap = np.concatenate([q_scale[HD // 2:], q_scale[:HD // 2]])
    ks_swap = np.concatenate([k_scale[HD // 2:], k_scale[:HD // 2]])
    cosq = np.ascontiguousarray(np.tile(cos * q_scale[None, :], (1, GS)))
    sinq = np.ascontiguousarray(np.tile(sin * (sgn * qs_swap)[None, :], (1, GS)))
    cosk = np.ascontiguousarray(cos * k_scale[None, :])
    sink = np.ascontiguousarray(sin * (sgn * ks_swap)[None, :])
    # diagonal-band mask, key-major: 1.0 where key j' may attend query i'
    maskd = np.ascontiguousarray((~mask[:CHUNK, :CHUNK]).T.astype(f32))
    ident = np.eye(128, dtype=f32)
    ones_col = np.ones((128, 1), f32)
    ones_row = np.ones((1, 128), f32)

    xTs = [np.ascontiguousarray(x[b].T) for b in range(B)]
    in_maps = []
    for c in range(NCORES):
        b, g = divmod(c, G)
        hs = slice(g * GS * HD, (g + 1) * GS * HD)
        gs = slice(g * HD, (g + 1) * HD)
        in_maps.append({
            "xT": xTs[b],
            "wq": np.ascontiguousarray(Wq[:, hs].astype(f32)),
            "wkv": np.ascontiguousarray(
                np.concatenate([Wk[:, gs], Wv[:, gs]], axis=1).astype(f32)),
            "wo": np.ascontiguousarray(Wo[:, hs].astype(f32)),
            "cosq": cosq, "sinq": sinq, "cosk": cosk, "sink": sink,
            "maskd": maskd, "ident": ident,
            "ones_col": ones_col, "ones_row": ones_row,
        })

    res = run_bass_kernel_spmd(nc, in_maps, list(range(NCORES)))
    LAST_RESULT = res

    out = np.empty((B, L, D), f32)
    for c in range(NCORES):
        b, g = divmod(c, G)
        out[b, :, g * CHUNK:(g + 1) * CHUNK] = res.results[c]["out"]
    return out

